# revision 1
# baseline (speedup 1.0000x reference)
"""MultiHeadAttention TRN2 Bass kernel.

Problem: B=16, L=1024, F=512, H=8 heads, D=64.
  q = Q@Wq+bq; k = K@Wk+bk; v = V@Wv+bv   (per-head split)
  S = q k^T / sqrt(D); P = softmax(S, axis=k); ctx = P v
  out = tanh(concat([ctx, Q]) @ Wo + bo)

Sharding: data-parallel over batch, 2 batches per core, 8 cores. No
collectives needed; full inputs sharded host-side, outputs gathered.

Device math (per core, fp16 compute / fp32 accumulate):
  - bk dropped entirely: its score contribution is constant along the
    softmax axis. bv folded into bo_eff = bo + bv @ Wo[:F] host-side.
  - QT/KT/VT loaded via XBAR DMA-transpose (fp16).
  - qT = Wq^T-proj(QT) + bq (per-partition ACT bias on the
    PSUM->SBUF copy);
    kT = Wk^T-proj(KT); v natural via VT-stationary matmul, stored with
    an appended ones column (v_aug = [v|1]) so the context matmul also
    yields the softmax denominator as PSUM row 64 for free.
  - scores computed transposed: S^T[k, q] = kT_h^T @ qT_h (K=64, head
    slices at partition bases 0/64), exp on ACT (scale=1/8) straight
    out of PSUM into fp16 SBUF.
  - ctxU^T[d, q] + denom = v_aug^T @ expS^T accumulated over k chunks.
  - normalization: recip(denom) on DVE (fp16 out, values <= 1),
    partition-broadcast via a K=1 ones matmul, one DVE multiply per
    head (division folds out of the k-sum, so it happens once on the
    small ctx, never on the big P matrix).
  - out[qtile, :] = tanh( sum_h ctxT_h^T Wo_h + Q Wo_bot + bo_eff ),
    bo_eff added via a K=1 ones-row matmul into the same accumulation.
"""

import numpy as np

import concourse.bass as bass
import concourse.tile as tile
from concourse import bacc, mybir
from concourse import bass_utils

B, L, F, H, D = 16, 1024, 512, 8, 64
NCORES = 8
BPC = B // NCORES  # batches per core
NFC = F // 128     # feature chunks (4)
NST = L // 128     # seq tiles (8)
F16 = mybir.dt.float16
F32 = mybir.dt.float32

MM_N = 512        # moving free dim per matmul (PSUM bank limit)
PSA_BUFS = 2
PSB_BUFS = 2
EXP_BUFS = 2
KVT_BUFS = 2
MISC_BUFS = 2
SC_ALT = True
SC_MOD = 8
SC_POS = 7
SC_POSSET = (7,)
PROJ_ALT = False
SC_HMIN = 0
PSR_POOL = "B"
PROJ_POOL = "B"
OUTP_POOL = "A"

_CACHE = {}


def _build_program():
    nc = bacc.Bacc("TRN2", target_bir_lowering=False)

    dQ = nc.dram_tensor("q_in", (BPC, L, F), F16, kind="ExternalInput")
    dK = nc.dram_tensor("k_in", (BPC, L, F), F16, kind="ExternalInput")
    dV = nc.dram_tensor("v_in", (BPC, L, F), F16, kind="ExternalInput")
    dWq = nc.dram_tensor("wq", (F, F), F16, kind="ExternalInput")
    dWk = nc.dram_tensor("wk", (F, F), F16, kind="ExternalInput")
    dWv = nc.dram_tensor("wv", (F, F), F16, kind="ExternalInput")
    dWoT = nc.dram_tensor("wo_top", (F, F), F16, kind="ExternalInput")
    dWoB = nc.dram_tensor("wo_bot", (F, F), F16, kind="ExternalInput")
    dbq = nc.dram_tensor("bq_cols", (128, NFC), F32, kind="ExternalInput")
    dbo = nc.dram_tensor("bo_eff", (1, F), F16, kind="ExternalInput")
    dOut = nc.dram_tensor("out", (BPC, L, F), F32, kind="ExternalOutput")

    with tile.TileContext(nc) as tc:
        _kernel(tc, dQ, dK, dV, dWq, dWk, dWv, dWoT, dWoB, dbq, dbo, dOut)

    nc.compile()
    return nc


def _kernel(tc, dQ, dK, dV, dWq, dWk, dWv, dWoT, dWoB, dbq, dbo, dOut):
    nc = tc.nc
    Exp = mybir.ActivationFunctionType.Exp
    Tanh = mybir.ActivationFunctionType.Tanh

    from contextlib import ExitStack
    ctx = ExitStack()
    consts = ctx.enter_context(tc.tile_pool(name="consts", bufs=1))
    p_qt = ctx.enter_context(tc.tile_pool(name="p_qt", bufs=2))
    p_kvt = ctx.enter_context(tc.tile_pool(name="p_kvt", bufs=KVT_BUFS))
    p_proj = ctx.enter_context(tc.tile_pool(name="p_proj", bufs=2))
    p_exp = ctx.enter_context(tc.tile_pool(name="p_exp", bufs=EXP_BUFS))
    p_ctx = ctx.enter_context(tc.tile_pool(name="p_ctx", bufs=2))
    p_misc = ctx.enter_context(tc.tile_pool(name="p_misc", bufs=MISC_BUFS))
    p_out = ctx.enter_context(tc.tile_pool(name="p_out", bufs=3))
    psA = ctx.enter_context(tc.tile_pool(name="psA", bufs=PSA_BUFS, space="PSUM"))
    psB = ctx.enter_context(tc.tile_pool(name="psB", bufs=PSB_BUFS, space="PSUM"))

    # ---- per-batch input transposes first: the first projection waits
    # on these, so issue them before the bulk of the weights.
    QTs, KTs, VTs = [], [], []
    for b in range(BPC):
        QT = p_qt.tile([128, NFC, L], F16, tag="QT")
        KT = p_kvt.tile([128, NFC, L], F16, tag="KT")
        VT = p_kvt.tile([128, NFC, L], F16, tag="VT")
        QTs.append(QT); KTs.append(KT); VTs.append(VT)

    for c in range(NFC):
        nc.sync.dma_start(out=QTs[0][:, c, :], in_=dQ[0, :, c * 128:(c + 1) * 128],
                          transpose=True)
    Wq_sb = consts.tile([128, NFC, F], F16, tag="wq")
    Wk_sb = consts.tile([128, NFC, F], F16, tag="wk")
    Wv_sb = consts.tile([128, NFC, F], F16, tag="wv")
    for c in range(NFC):
        nc.sync.dma_start(out=Wq_sb[:, c, :], in_=dWq[c * 128:(c + 1) * 128, :])
    bq_sb = consts.tile([128, NFC], F32, tag="bq")
    nc.sync.dma_start(out=bq_sb, in_=dbq[:, :])
    ones_sb = consts.tile([128, 1024], F16, tag="ones")
    nc.vector.memset(ones_sb, 1.0)
    for c in range(NFC):
        nc.sync.dma_start(out=KTs[0][:, c, :], in_=dK[0, :, c * 128:(c + 1) * 128],
                         transpose=True)
        nc.sync.dma_start(out=VTs[0][:, c, :], in_=dV[0, :, c * 128:(c + 1) * 128],
                         transpose=True)
    for c in range(NFC):
        nc.sync.dma_start(out=Wk_sb[:, c, :], in_=dWk[c * 128:(c + 1) * 128, :])
        nc.sync.dma_start(out=Wv_sb[:, c, :], in_=dWv[c * 128:(c + 1) * 128, :])
    # Wo top half per-head at partition base 0; bottom in 128-row chunks
    WoT_sb = consts.tile([128, H, F], F16, tag="wot")
    for h in range(H):
        nc.sync.dma_start(out=WoT_sb[0:D, h, :], in_=dWoT[h * D:(h + 1) * D, :])
    WoB_sb = consts.tile([128, NFC, F], F16, tag="wob")
    for c in range(NFC):
        nc.sync.dma_start(out=WoB_sb[:, c, :], in_=dWoB[c * 128:(c + 1) * 128, :])
    bo_sb = consts.tile([1, F], F16, tag="bo")
    nc.sync.dma_start(out=bo_sb, in_=dbo[0:1, :])
    for b in range(1, BPC):
        for c in range(NFC):
            nc.sync.dma_start(out=QTs[b][:, c, :],
                              in_=dQ[b, :, c * 128:(c + 1) * 128], transpose=True)
            nc.sync.dma_start(out=KTs[b][:, c, :],
                             in_=dK[b, :, c * 128:(c + 1) * 128], transpose=True)
            nc.sync.dma_start(out=VTs[b][:, c, :],
                             in_=dV[b, :, c * 128:(c + 1) * 128], transpose=True)

    NH = L // MM_N  # moving-dim chunks per full seq (1 when MM_N=1024)

    for b in range(BPC):
        QT, KT, VT = QTs[b], KTs[b], VTs[b]

        # ---- projections ------------------------------------------
        qT = p_proj.tile([128, NFC, L], F16, tag="qT")
        kT = p_proj.tile([128, NFC, L], F16, tag="kT")
        # per head: [v(64) | one] -> ctx rows 0:64, denom row 64
        vaug = p_proj.tile([128, NST, H, D + 1], F16, tag="vaug")
        nc.vector.memset(vaug[:, :, :, D:D + 1], 1.0)

        for fo in range(NFC):
            pp = "A" if (PROJ_ALT and fo == NFC - 1) else PROJ_POOL
            ps = (psA if pp == "A" else psB).tile([128, 1024], F32, tag=pp)
            for nh in range(NH):
                o = ps[:, nh * MM_N:(nh + 1) * MM_N]
                for c in range(NFC):
                    nc.tensor.matmul(o, Wq_sb[:, c, fo * 128:(fo + 1) * 128],
                                     QT[:, c, nh * MM_N:(nh + 1) * MM_N],
                                     start=(c == 0), stop=(c == NFC - 1))
            nc.scalar.activation(out=qT[:, fo, :], in_=ps,
                                 func=mybir.ActivationFunctionType.Identity,
                                 bias=bq_sb[:, fo:fo + 1], scale=1.0)

        for fo in range(NFC):
            ps = (psA if PROJ_POOL == "A" else psB).tile([128, 1024], F32, tag=PROJ_POOL)
            for nh in range(NH):
                o = ps[:, nh * MM_N:(nh + 1) * MM_N]
                for c in range(NFC):
                    nc.tensor.matmul(o, Wk_sb[:, c, fo * 128:(fo + 1) * 128],
                                     KT[:, c, nh * MM_N:(nh + 1) * MM_N],
                                     start=(c == 0), stop=(c == NFC - 1))
            nc.vector.tensor_copy(out=kT[:, fo, :], in_=ps)

        for st in range(NST):
            ps = (psA if PROJ_POOL == "A" else psB).tile([128, 1024], F32, tag=PROJ_POOL)
            for c in range(NFC):
                nc.tensor.matmul(ps[:, 0:512], VT[:, c, st * 128:(st + 1) * 128],
                                 Wv_sb[:, c, :], start=(c == 0), stop=(c == NFC - 1))
            nc.vector.tensor_copy(
                out=vaug[:, st, :, 0:D],
                in_=ps[:, 0:512].rearrange("p (h d) -> p h d", h=H))

        # ---- attention --------------------------------------------
        # ctxT: one head per chunk, partitions 0:64 (normalized, fp16)
        ctxT = p_ctx.tile([128, H, L], F16, tag="ctxT")
        for h in range(H):
            hb = (h % 2) * 64
            hc = h // 2
            hc2 = h
            expS = p_exp.tile([128, NST, L], F16, tag="expS")
            for kt in range(NST):
                if SC_ALT and (kt % SC_MOD) in SC_POSSET and h >= SC_HMIN:
                    ps = psB.tile([128, 1024], F32, tag="B")
                else:
                    ps = psA.tile([128, 1024], F32, tag="A")
                for nh in range(NH):
                    nc.tensor.matmul(
                        ps[:, nh * MM_N:(nh + 1) * MM_N],
                        kT[hb:hb + D, hc, kt * 128:(kt + 1) * 128],
                        qT[hb:hb + D, hc, nh * MM_N:(nh + 1) * MM_N],
                        start=True, stop=True)
                nc.scalar.activation(out=expS[:, kt, :], in_=ps, func=Exp,
                                     scale=0.125)

            psc = psB.tile([128, 1024], F32, tag="B")
            for nh in range(NH):
                o = psc[0:D + 1, nh * MM_N:(nh + 1) * MM_N]
                for kt in range(NST):
                    nc.tensor.matmul(o, vaug[:, kt, h, :],
                                     expS[:, kt, nh * MM_N:(nh + 1) * MM_N],
                                     start=(kt == 0), stop=(kt == NST - 1))
            recip = p_misc.tile([128, 1024], F16, tag="recip")
            psr = (psA if PSR_POOL == "A" else psB).tile([128, 1024], F32, tag=PSR_POOL)
            for half in range(2):
                sl = slice(half * 512, (half + 1) * 512)
                with nc.allow_low_precision(reason="softmax recip <=1, fp16 ok"):
                    nc.vector.reciprocal(out=recip[D:D + 1, sl],
                                         in_=psc[D:D + 1, sl])
                nc.tensor.matmul(psr[0:D, sl], ones_sb[D:D + 1, 0:D],
                                 recip[D:D + 1, sl], start=True, stop=True)
                nc.vector.tensor_copy(out=ctxT[0:D, hc2, sl], in_=psc[0:D, sl])
                nc.vector.tensor_mul(out=ctxT[0:D, hc2, sl], in0=psr[0:D, sl],
                                     in1=ctxT[0:D, hc2, sl])

        # ---- output projection ------------------------------------
        for qt in range(NST):
            ps = (psA if OUTP_POOL == "A" else psB).tile([128, 1024], F32, tag=OUTP_POOL)
            o = ps[:, 0:512]
            for h in range(H):
                nc.tensor.matmul(o, ctxT[0:D, h, qt * 128:(qt + 1) * 128],
                                 WoT_sb[0:D, h, :], start=(h == 0), stop=False)
            for c in range(NFC):
                nc.tensor.matmul(o, QT[:, c, qt * 128:(qt + 1) * 128],
                                 WoB_sb[:, c, :], start=False, stop=False)
            nc.tensor.matmul(o, ones_sb[0:1, 0:128],
                             bo_sb[0:1, :], start=False, stop=True)
            out_t = p_out.tile([128, 512], F32, tag="out")
            nc.scalar.activation(out=out_t, in_=o, func=Tanh)
            nc.sync.dma_start(out=dOut[b, qt * 128:(qt + 1) * 128, :], in_=out_t)

    ctx.close()


def kernel(Q, K, V, Wq, bq, Wk, bk, Wv, bv, Wo, bo):
    if "nc" not in _CACHE:
        _CACHE["nc"] = _build_program()
    nc = _CACHE["nc"]

    Q, K, V = (np.asarray(x, dtype=np.float32) for x in (Q, K, V))
    Wq, Wk, Wv, Wo = (np.asarray(x, dtype=np.float32) for x in (Wq, Wk, Wv, Wo))
    bq, bv, bo = (np.asarray(x, dtype=np.float32) for x in (bq, bv, bo))

    f32 = np.float32
    Wo_top = Wo[:F, :].astype(f32)
    bo_eff = bo.astype(f32) + bv.astype(f32) @ Wo_top  # bv folds through Wo
    h16 = np.float16

    in_common = {
        "wq": Wq.astype(h16), "wk": Wk.astype(h16), "wv": Wv.astype(h16),
        "wo_top": Wo_top.astype(h16), "wo_bot": Wo[F:, :].astype(h16),
        "bq_cols": np.ascontiguousarray(bq.reshape(NFC, 128).T).astype(f32),
        "bo_eff": bo_eff.reshape(1, F).astype(h16),
    }
    Qh = np.asarray(Q, dtype=h16)
    Kh = np.asarray(K, dtype=h16)
    Vh = np.asarray(V, dtype=h16)
    in_maps = []
    for c in range(NCORES):
        s = slice(c * BPC, (c + 1) * BPC)
        in_maps.append({"q_in": Qh[s], "k_in": Kh[s], "v_in": Vh[s], **in_common})

    _CACHE["in_maps"] = in_maps
    res = bass_utils.run_bass_kernel_spmd(nc, in_maps, core_ids=list(range(NCORES)))
    out = np.concatenate([r["out"] for r in res.results], axis=0)
    return out.astype(np.float32)


def _last_in_maps():
    return _CACHE["in_maps"]



# revision 16
# speedup vs baseline: 1.3334x; 1.3334x over previous
"""MultiHeadAttention TRN2 Bass kernel (fp8-DoubleRow + dual-engine exp).

Problem: B=16, L=1024, F=512, H=8 heads, D=64.
  q = Q@Wq+bq; k = K@Wk+bk; v = V@Wv+bv   (per-head split)
  S = q k^T / sqrt(D); P = softmax(S, axis=k); ctx = P v
  out = tanh(concat([ctx, Q]) @ Wo + bo)

Sharding: data-parallel over batch, 2 batches per core, 8 cores.

Device strategy (cost-model driven):
  - All projection / score / context matmuls run fp8e4m3 with
    MatmulPerfMode.DoubleRow (0.5 cyc/output-col, K=2 planes/instr).
  - Inputs Q/K/V are quantized to e4m3 on host and feature-pairs packed
    into uint16 so the XBAR DMA-transpose (2-byte only) can load them;
    on SBUF an interleaved-plane fp8 AP view feeds DoubleRow directly.
  - Scores are computed pre-scaled by A16 = 2^10/ln2 (folded into the
    q/k quantization scales) so that:
      * ACT-engine path: exp via activation, scale=1/A16, bias=-SHIFT
      * DVE-engine path: Schraudolph bit-trick exp: int16(S~ + Boff)
        bitcast to fp16 IS approximately exp(S - SHIFT); softmax is
        shift-invariant so the common SHIFT cancels.
    exp work is split between ACT (k-tiles 0..3 -> fp8 P) and DVE
    (k-tiles 4..7 -> fp16 P) to balance the two engines.
  - Context layout B: ctx[q, h, d] with an appended ones column on v
    giving the softmax denominator for free; normalization by 1/Z on
    DVE with a broadcast AP; result written fp8 (x16 scale).
  - ctx is PE-transposed (fp8 identity matmul) and DMA-evacuated so the
    output projection can contract hd with K=128/K=256-DR.
  - Output projection is TRANSPOSED: outT[f, q] so bo folds into the
    per-partition tanh bias; skip path Q@WoB stays fp16 (precision),
    ctx@WoT is fp8-DR.  Host undoes the transpose (free).
  - Weight/ctx scale folding: Wq,Wk,Wv x32 (e4m3 normal range), ctx x16,
    WoT x16, Q x16, WoB x16; tanh activation scale = 1/256.
"""

import numpy as np
import ml_dtypes

import concourse.bass as bass
import concourse.tile as tile
from concourse import bacc, mybir
from concourse import bass_utils

B, L, F, H, D = 16, 1024, 512, 8, 64
NCORES = 8
BPC = B // NCORES
NST = L // 128     # seq tiles (8)
F16 = mybir.dt.float16
F32 = mybir.dt.float32
FP8 = mybir.dt.float8e4
U16 = mybir.dt.uint16
I16 = mybir.dt.int16

A16 = 1024.0 / np.log(2.0)        # 1477.3196
ALPHA = float(np.sqrt(A16 / 8.0)) # q/k fp8 scale so psum S~ = A16 * S_true
SHIFT = 2.5                       # exp(S - SHIFT): keeps e4m3 P in range
B16_BASE = 15360.0                # fp16 Schraudolph bias
B16_TUNE = 22.0                   # truncation/centering correction
W_SCALE = 32.0                    # Wq/Wk/Wv fp8 scale
CTX_SCALE = 16.0                  # ctx fp8 scale
WO_SCALE = 16.0                   # WoT fp8 / WoB,Q fp16 scale
NKT8 = 4                          # k-tiles 0..NKT8-1 on ACT (fp8 P)

_CACHE = {}

E4M3 = ml_dtypes.float8_e4m3


def _build_program():
    nc = bacc.Bacc("TRN2", target_bir_lowering=False)

    dQ16 = nc.dram_tensor("q16", (BPC, L, F), F16, kind="ExternalInput")
    dQ8 = nc.dram_tensor("q8p", (BPC, L // 2, F), U16, kind="ExternalInput")
    dK8 = nc.dram_tensor("k8p", (BPC, L // 2, F), U16, kind="ExternalInput")
    dV8 = nc.dram_tensor("v8p", (BPC, L // 2, F), U16, kind="ExternalInput")
    dWq = nc.dram_tensor("wq8", (128, 2 * 2 * F), FP8, kind="ExternalInput")
    dWk = nc.dram_tensor("wk8", (128, 2 * 2 * F), FP8, kind="ExternalInput")
    dWv = nc.dram_tensor("wv8", (128, 2 * 2 * F), FP8, kind="ExternalInput")
    dWoT = nc.dram_tensor("wot8", (128, 2 * 2 * F), FP8, kind="ExternalInput")
    dWoB = nc.dram_tensor("wob16", (128, 4 * F), F16, kind="ExternalInput")
    dBq = nc.dram_tensor("bq_p", (128, 4), F32, kind="ExternalInput")
    dBo = nc.dram_tensor("bo_p", (128, 4), F32, kind="ExternalInput")
    dId = nc.dram_tensor("ident8", (128, 128), F16, kind="ExternalInput")
    dOut = nc.dram_tensor("outT", (BPC, F, L), F16, kind="ExternalOutput")
    dDbgV8 = nc.dram_tensor("dbg_v8", (128, 2 * 2 * H * (D + 1)), FP8, kind="ExternalOutput")
    dDbgV16 = nc.dram_tensor("dbg_v16", (128, 4 * H * (D + 1)), F16, kind="ExternalOutput")
    dDbgCN = nc.dram_tensor("dbg_cn", (128, NST * H * D), F16, kind="ExternalOutput")
    dDbgCT = nc.dram_tensor("dbg_ct", (128, 2 * 2 * L), FP8, kind="ExternalOutput")
    dDbgE8 = nc.dram_tensor("dbg_e8", (128, 2 * 2 * L), FP8, kind="ExternalOutput")

    with tile.TileContext(nc) as tc:
        _kernel(tc, dQ16, dQ8, dK8, dV8, dWq, dWk, dWv, dWoT, dWoB,
                dBq, dBo, dId, dOut,
                (dDbgV8, dDbgV16, dDbgCN, dDbgCT, dDbgE8))

    nc.compile()
    return nc


def _kernel(tc, dQ16, dQ8, dK8, dV8, dWq, dWk, dWv, dWoT, dWoB,
            dBq, dBo, dId, dOut, dbg=None):
    nc = tc.nc
    Exp = mybir.ActivationFunctionType.Exp
    Tanh = mybir.ActivationFunctionType.Tanh
    Ident = mybir.ActivationFunctionType.Identity
    Copy = mybir.ActivationFunctionType.Copy
    Mult = mybir.AluOpType.mult
    Add = mybir.AluOpType.add
    DR = mybir.MatmulPerfMode.DoubleRow

    from contextlib import ExitStack
    ctx = ExitStack()
    consts = ctx.enter_context(tc.tile_pool(name="consts", bufs=1))
    p_in = ctx.enter_context(tc.tile_pool(name="p_in", bufs=2))
    p_qk = ctx.enter_context(tc.tile_pool(name="p_qk", bufs=2))
    p_va = ctx.enter_context(tc.tile_pool(name="p_va", bufs=2))
    p_exp = ctx.enter_context(tc.tile_pool(name="p_exp", bufs=2))
    p_cn = ctx.enter_context(tc.tile_pool(name="p_cn", bufs=2))
    p_out = ctx.enter_context(tc.tile_pool(name="p_out", bufs=2))
    p_rc = ctx.enter_context(tc.tile_pool(name="p_rc", bufs=2))
    psA = ctx.enter_context(tc.tile_pool(name="psA", bufs=3, space="PSUM"))
    psC = ctx.enter_context(tc.tile_pool(name="psC", bufs=2, space="PSUM"))

    # ---------------- per-batch input loads (first batch first) --------
    QT16s, Q8s, K8s, V8s = [], [], [], []
    for b in range(BPC):
        QT16s.append(p_in.tile([128, 4, L], F16, tag="qt16", name=f"qt16_{b}"))
        Q8s.append(p_in.tile([128, 4, L // 2], U16, tag="q8u", name=f"q8u_{b}"))
        K8s.append(p_in.tile([128, 4, L // 2], U16, tag="k8u", name=f"k8u_{b}"))
        V8s.append(p_in.tile([128, 4, L // 2], U16, tag="v8u", name=f"v8u_{b}"))

    def load_batch(b):
        for c in range(4):
            nc.sync.dma_start(out=Q8s[b][:, c, :],
                              in_=dQ8[b, :, c * 128:(c + 1) * 128],
                              transpose=True)
            nc.sync.dma_start(out=K8s[b][:, c, :],
                              in_=dK8[b, :, c * 128:(c + 1) * 128],
                              transpose=True)
            nc.sync.dma_start(out=V8s[b][:, c, :],
                              in_=dV8[b, :, c * 128:(c + 1) * 128],
                              transpose=True)
        for c in range(4):
            nc.sync.dma_start(out=QT16s[b][:, c, :],
                              in_=dQ16[b, :, c * 128:(c + 1) * 128],
                              transpose=True)

    load_batch(0)

    # ---------------- weights / consts ---------------------------------
    Wq_sb = consts.tile([128, 2, 2, F], FP8, tag="wq")
    Wk_sb = consts.tile([128, 2, 2, F], FP8, tag="wk")
    Wv_sb = consts.tile([128, 2, 2, F], FP8, tag="wv")
    WoT_sb = consts.tile([128, 2, 2, F], FP8, tag="wot")
    WoB_sb = consts.tile([128, 4, F], F16, tag="wob")
    bq_sb = consts.tile([128, 4], F32, tag="bq")
    bo_sb = consts.tile([128, 4], F32, tag="bo")
    id_sb = consts.tile([128, 128], F16, tag="id")
    nshift_sb = consts.tile([128, 1], F32, tag="nshift")
    nc.vector.memset(nshift_sb, -SHIFT)
    nc.sync.dma_start(out=Wq_sb, in_=dWq[:, :].rearrange(
        "p (c i f) -> p c i f", c=2, i=2))
    nc.sync.dma_start(out=Wk_sb, in_=dWk[:, :].rearrange(
        "p (c i f) -> p c i f", c=2, i=2))
    nc.sync.dma_start(out=Wv_sb, in_=dWv[:, :].rearrange(
        "p (c i f) -> p c i f", c=2, i=2))
    nc.sync.dma_start(out=WoT_sb, in_=dWoT[:, :].rearrange(
        "p (c i f) -> p c i f", c=2, i=2))
    nc.sync.dma_start(out=WoB_sb, in_=dWoB[:, :].rearrange(
        "p (c f) -> p c f", c=4))
    nc.sync.dma_start(out=bq_sb, in_=dBq[:, :])
    nc.sync.dma_start(out=bo_sb, in_=dBo[:, :])
    nc.sync.dma_start(out=id_sb, in_=dId[:, :])

    if BPC > 1:
        load_batch(1)

    for b in range(BPC):
        QT16, Q8u, K8u, V8u = QT16s[b], Q8s[b], K8s[b], V8s[b]
        # fp8 interleaved-plane views of the paired inputs:
        # [128, c, 2*L]u16 -> fp8 [128, c, L, 2] -> AP [128, c, 2(i), L]
        Q8 = Q8u.bitcast(FP8)
        K8 = K8u.bitcast(FP8)
        V8 = V8u.bitcast(FP8)

        # ---- projections (fp8 DoubleRow) ---------------------------
        # q/k: out chunks are sigma-permuted hd (score-ready layout)
        qs8 = p_qk.tile([128, 2, 2, L], FP8, tag="qs8")
        ks8 = p_qk.tile([128, 2, 2, L], FP8, tag="ks8")
        for ch in range(4):
            g, pl = ch // 2, ch % 2
            ps = psA.tile([128, 1024], F32, tag="A")
            for nh in range(2):
                o = ps[:, nh * 512:(nh + 1) * 512]
                for u in range(2):
                    nc.tensor.matmul(
                        o, Wq_sb[:, u, :, ch * 128:(ch + 1) * 128],
                        Q8[:, 2 * u:2 * u + 2, nh * 512:(nh + 1) * 512],
                        start=(u == 0), stop=(u == 1), perf_mode=DR)
            nc.scalar.activation(out=qs8[:, g, pl, :], in_=ps, func=Ident,
                                 bias=bq_sb[:, ch:ch + 1],
                                 scale=ALPHA / W_SCALE)
        for ch in range(4):
            g, pl = ch // 2, ch % 2
            ps = psA.tile([128, 1024], F32, tag="A")
            for nh in range(2):
                o = ps[:, nh * 512:(nh + 1) * 512]
                for u in range(2):
                    nc.tensor.matmul(
                        o, Wk_sb[:, u, :, ch * 128:(ch + 1) * 128],
                        K8[:, 2 * u:2 * u + 2, nh * 512:(nh + 1) * 512],
                        start=(u == 0), stop=(u == 1), perf_mode=DR)
            nc.scalar.activation(out=ks8[:, g, pl, :], in_=ps, func=Copy,
                                 scale=ALPHA / W_SCALE)

        # v: natural [k-tile, hd]; vaug8 for kt<NKT8, vaug16 for the rest
        vaug8 = p_va.tile([128, 2, 2, H, D + 1], FP8, tag="v8")
        vaug16 = p_va.tile([128, 4, H, D + 1], F16, tag="v16")
        nc.vector.memset(vaug8[:, :, :, :, D:D + 1], 1.0)
        nc.vector.memset(vaug16[:, :, :, D:D + 1], 1.0)
        for kt in range(NST):
            ps = psA.tile([128, 512], F32, tag="A")
            for u in range(2):
                nc.tensor.matmul(
                    ps, V8[:, 2 * u:2 * u + 2, kt * 128:(kt + 1) * 128],
                    Wv_sb[:, u, :, :],
                    start=(u == 0), stop=(u == 1), perf_mode=DR)
            src = ps[:, 0:512].rearrange("p (h d) -> p h d", h=H)
            if kt < NKT8:
                nc.scalar.activation(
                    out=vaug8[:, kt // 2, kt % 2, :, 0:D], in_=src,
                    func=Copy, scale=1.0 / W_SCALE)
            else:
                nc.vector.tensor_scalar(
                    out=vaug16[:, kt - NKT8, :, 0:D], in0=src,
                    scalar1=1.0 / W_SCALE, scalar2=None, op0=Mult)

        if b == 0 and dbg is not None:
            nc.sync.dma_start(out=dbg[0][:, :].rearrange('p (t u h d) -> p t u h d', t=2, u=2, h=H), in_=vaug8[:, :, :, :, :])
            nc.sync.dma_start(out=dbg[1][:, :].rearrange('p (t h d) -> p t h d', t=4, h=H), in_=vaug16[:, :, :, :])

        # ---- attention ---------------------------------------------
        ctx_n = p_cn.tile([128, NST, H, D], F16, tag="ctxn")
        boff = float(B16_BASE + B16_TUNE - SHIFT * A16)
        for h in range(H):
            pb, gg = 32 * (h % 4), h // 4
            expS8 = p_exp.tile([128, 2, 2, L], FP8, tag="e8")
            expS16 = p_exp.tile([128, 4, L], I16, tag="e16")
            for kt in range(NST):
                ps = psA.tile([128, 1024], F32, tag="A")
                for nh in range(2):
                    nc.tensor.matmul(
                        ps[:, nh * 512:(nh + 1) * 512],
                        ks8[pb:pb + 32, gg, :, kt * 128:(kt + 1) * 128],
                        qs8[pb:pb + 32, gg, :, nh * 512:(nh + 1) * 512],
                        start=True, stop=True, perf_mode=DR,
                        tile_position=(pb, 0))
                if kt < NKT8:
                    nc.scalar.activation(
                        out=expS8[:, kt // 2, kt % 2, :], in_=ps,
                        func=Exp, scale=1.0 / A16, bias=nshift_sb[:, 0:1])
                else:
                    nc.vector.tensor_scalar(
                        out=expS16[:, kt - NKT8, :], in0=ps,
                        scalar1=boff, scalar2=None, op0=Add)
            if b == 0 and h == 0 and dbg is not None:
                nc.sync.dma_start(out=dbg[4][:, :].rearrange('p (a b n) -> p a b n', a=2, b=2), in_=expS8[:, :, :, :])
            expS16f = expS16.bitcast(F16)
            for grp in range(2):
                cps = psC.tile([128, 4, D + 1], F32, tag="C")
                first = True
                for qi in range(4):
                    qt = grp * 4 + qi
                    o = cps[:, qi, :]
                    for t in range(2):
                        nc.tensor.matmul(
                            o, expS8[:, t, :, qt * 128:(qt + 1) * 128],
                            vaug8[:, t, :, h, :],
                            start=first, stop=False, perf_mode=DR)
                        first = False
                    for kt in range(NKT8, NST):
                        last = (qi == 3) and (kt == NST - 1)
                        nc.tensor.matmul(
                            o, expS16f[:, kt - NKT8, qt * 128:(qt + 1) * 128],
                            vaug16[:, kt - NKT8, h, :],
                            start=False, stop=last)
                rc = p_rc.tile([128, 4], F32, tag="rc")
                nc.vector.reciprocal(out=rc, in_=cps[:, :, D])
                rcb = rc[:, :].unsqueeze(2).broadcast_to([128, 4, D])
                nc.vector.scalar_tensor_tensor(
                    out=ctx_n[:, grp * 4:grp * 4 + 4, h, :],
                    in0=cps[:, :, 0:D], scalar=CTX_SCALE, in1=rcb,
                    op0=Mult, op1=Mult)

        # ---- transpose ctx -> ctxT8 [hd, q] ------------------------
        ctxT8 = p_cn.tile([128, 2, 2, L], FP8, tag="ctxT")
        for qt in range(NST):
            tps = psC.tile([128, 4, 128], F16, tag="C")
            for c in range(4):
                nc.tensor.transpose(
                    tps[:, c, :],
                    ctx_n[:, qt, 2 * c:2 * c + 2, :], id_sb)
            dst = ctxT8[:, :, :, qt * 128:(qt + 1) * 128]
            src = tps[:, :, :].rearrange("p (a b) n -> p a b n", a=2)
            if qt % 2 == 0:
                nc.vector.tensor_copy(out=dst, in_=src)
            else:
                nc.scalar.copy(out=dst, in_=src)

        if b == 0 and dbg is not None:
            nc.sync.dma_start(out=dbg[2][:, :].rearrange('p (q h d) -> p q h d', q=NST, h=H), in_=ctx_n[:, :, :, :])
            nc.sync.dma_start(out=dbg[3][:, :].rearrange('p (a b n) -> p a b n', a=2, b=2), in_=ctxT8[:, :, :, :])

        # ---- output projection (transposed: outT[f, q]) ------------
        outT = p_out.tile([128, 4, L], F16, tag="outT")
        for fo in range(4):
            ps = psA.tile([128, 1024], F32, tag="A")
            for nh in range(2):
                o = ps[:, nh * 512:(nh + 1) * 512]
                for cp in range(2):
                    nc.tensor.matmul(
                        o, WoT_sb[:, cp, :, fo * 128:(fo + 1) * 128],
                        ctxT8[:, cp, :, nh * 512:(nh + 1) * 512],
                        start=(cp == 0), stop=False, perf_mode=DR)
                for c in range(4):
                    nc.tensor.matmul(
                        o, WoB_sb[:, c, fo * 128:(fo + 1) * 128],
                        QT16[:, c, nh * 512:(nh + 1) * 512],
                        start=False, stop=(c == 3))
            nc.scalar.activation(out=outT[:, fo, :], in_=ps, func=Tanh,
                                 bias=bo_sb[:, fo:fo + 1],
                                 scale=1.0 / (WO_SCALE * WO_SCALE))
            nc.sync.dma_start(out=dOut[b, fo * 128:(fo + 1) * 128, :],
                              in_=outT[:, fo, :])

    ctx.close()


def _host_prep(Wq, bq, Wk, Wv, Wo, bv, bo):
    """Host-side weight layout + scaling. Returns the common input map."""
    f32 = np.float32
    # sigma column permutation for q/k projections: chunk ch=(g,pl),
    # partition 32j+delta -> head 4g+j, d = 32 pl + delta
    colmap = np.empty(F, dtype=np.int64)
    for ch in range(4):
        g, pl = ch // 2, ch % 2
        for j in range(4):
            for dlt in range(32):
                m = ch * 128 + 32 * j + dlt
                colmap[m] = 64 * (4 * g + j) + 32 * pl + dlt

    def pack_w(Wmat, cmap=None, scale=W_SCALE):
        # [128, u(2), i(2), 512] with rows f = 128*(2u+i) + p
        Wp = Wmat if cmap is None else Wmat[:, cmap]
        out = np.empty((128, 2, 2, F), dtype=E4M3)
        for u in range(2):
            for i in range(2):
                base = 128 * (2 * u + i)
                out[:, u, i, :] = (scale * Wp[base:base + 128, :]).astype(E4M3)
        return out.reshape(128, 2 * 2 * F)

    WoT = Wo[:F, :].astype(f32)
    WoB = Wo[F:, :].astype(f32)
    bo_eff = bo.astype(f32) + bv.astype(f32) @ WoT  # bv folds through WoT

    # WoT8: [128, cp(2), cs(2), 512] rows hd = 128*(2cp+cs) + p
    wot8 = np.empty((128, 2, 2, F), dtype=E4M3)
    for cp in range(2):
        for cs in range(2):
            base = 128 * (2 * cp + cs)
            wot8[:, cp, cs, :] = (WO_SCALE *
                                  WoT[base:base + 128, :]).astype(E4M3)
    # WoB16: [128, c(4), 512] rows f = 128c + p
    wob16 = (WO_SCALE / WO_SCALE) * np.stack(
        [WoB[128 * c:128 * (c + 1), :] for c in range(4)], axis=1)
    wob16 = (wob16 * WO_SCALE).astype(np.float16)

    bq_p = np.ascontiguousarray(
        (ALPHA * bq.astype(f32))[colmap].reshape(4, 128).T).astype(f32)
    bo_p = np.ascontiguousarray(bo_eff.reshape(4, 128).T).astype(f32)

    return {
        "wq8": pack_w(Wq.astype(f32), colmap),
        "wk8": pack_w(Wk.astype(f32), colmap),
        "wv8": pack_w(Wv.astype(f32)),
        "wot8": wot8.reshape(128, 2 * 2 * F),
        "wob16": wob16.reshape(128, 4 * F),
        "bq_p": bq_p,
        "bo_p": bo_p,
        "ident8": np.eye(128, dtype=np.float16),
    }


def kernel(Q, K, V, Wq, bq, Wk, bk, Wv, bv, Wo, bo):
    if "nc" not in _CACHE:
        _CACHE["nc"] = _build_program()
    nc = _CACHE["nc"]

    f32 = np.float32
    Q, K, V = (np.asarray(x, dtype=f32) for x in (Q, K, V))

    in_common = _host_prep(np.asarray(Wq, f32), np.asarray(bq, f32),
                           np.asarray(Wk, f32), np.asarray(Wv, f32),
                           np.asarray(Wo, f32), np.asarray(bv, f32),
                           np.asarray(bo, f32))

    def pair8(X):  # (BPC, L, F) f32 -> (BPC, L/2, F) u16: k-position pairs
        nb = X.shape[0]
        X8 = X.astype(E4M3).reshape(nb, L // 2, 2, F).transpose(0, 1, 3, 2)
        return np.ascontiguousarray(X8).view(np.uint16).reshape(nb, L // 2, F)

    Q16 = (WO_SCALE * Q).astype(np.float16)
    Q8, K8, V8 = pair8(Q), pair8(K), pair8(V)

    in_maps = []
    for c in range(NCORES):
        s = slice(c * BPC, (c + 1) * BPC)
        in_maps.append({"q16": Q16[s], "q8p": Q8[s], "k8p": K8[s],
                        "v8p": V8[s], **in_common})

    _CACHE["in_maps"] = in_maps
    res = bass_utils.run_bass_kernel_spmd(nc, in_maps,
                                          core_ids=list(range(NCORES)))
    # outT is (BPC, F, L) fp16 -> (B, L, F) fp32
    out = np.concatenate(
        [r["outT"].transpose(0, 2, 1) for r in res.results], axis=0)
    return np.ascontiguousarray(out).astype(f32)


def _last_in_maps():
    return _CACHE["in_maps"]
